# revision 3
# baseline (speedup 1.0000x reference)
"""CNF step (3-layer tanh MLP vector field + exact divergence) on 8 trn2 cores.

Math: for each sample x in R^64 (x's last column is the logp channel, replaced
by scalar t in the MLP input):
    h1 = tanh([x, t] @ W1 + b1);  h2 = tanh(h1 @ W2 + b2)
    dx = (h2 @ W3 + b3) / 2
    div = trace(J) where J = d(dx)/dx
Closed form for the jacobian trace (avoids jacrev entirely):
    div = (1/2) * d1^T K d2,  d1 = 1-h1^2, d2 = 1-h2^2,
    K[m,j] = W2[m,j] * sum_i W1[i,m] W3[j,i]
K is a pure function of the (launch-invariant) weights, folded on host once.
All O(batch) compute runs on device.

Device layout is fully transposed (features on partitions, batch on free dim):
weights serve directly as matmul lhsT operands, so the kernel needs zero
on-device transposes. Host pre-transposes x (layout prep) and re-transposes
the output.

Sharding: pure data parallel, batch 2048 -> 8 cores x 256 samples.
"""

import numpy as np

import bass_rust
import concourse.bass as bass
import concourse.tile as tile
from concourse import mybir
from concourse.bass_utils import run_bass_kernel_spmd

# This walrus build only encodes a single sem-wait per instruction; Tile's
# scheduler freely emits instructions carrying 2-3 waits and codegen dies
# with "Too many sync wait commands". Two patches hoist extra waits onto
# single-wait EventSemaphore carrier instructions placed immediately before
# the multi-wait instruction on the same engine (semantically identical:
# engines execute in order, all waits still precede the op).
_orig_add_instruction = tile.TileContext._add_instruction


def _split_waits(tc_self, inst):
    si = getattr(inst, "sync_info", None)
    if (
        si is not None
        and si.on_wait
        and len(si.on_wait) > 1
        and inst.engine != mybir.EngineType.Unassigned
    ):
        waits = list(si.on_wait)
        upds = list(si.on_update) if si.on_update else []
        for w in waits[:-1]:
            carrier = mybir.InstEventSemaphore(
                name=tc_self.nc.get_next_instruction_name(),
                engine=inst.engine,
                ins=[],
                outs=[],
                sync_info=mybir.SyncInfo(on_wait=[w], on_update=[]),
                bass_nofuse=True,
            )
            _orig_add_instruction(tc_self, carrier)
        inst.sync_info = mybir.SyncInfo(on_wait=[waits[-1]], on_update=upds)


def _patched_add_instruction(self, inst):
    _split_waits(self, inst)
    _orig_add_instruction(self, inst)


tile.TileContext._add_instruction = _patched_add_instruction


# Same fix for the kernel-tail drain (emitted after lowering, outside
# _add_instruction): split it into a chain of single-wait drains.
def _patched_drain_and_barrier(self, tick_clock, wait_clock):
    nc = self.nc
    drain_inst = nc.sync.drain()
    wait_clock.add_sem_waits(
        drain_inst.ins, bass_rust.ScopedClock({None: tick_clock.global_clock})
    )
    si = drain_inst.ins.sync_info
    waits = list(si.on_wait) if si is not None and si.on_wait else []
    if len(waits) > 1:
        upds = list(si.on_update) if si.on_update else []
        drain_inst.ins.sync_info = mybir.SyncInfo(on_wait=[waits[0]], on_update=upds)
        for w in waits[1:]:
            extra = nc.sync.drain()
            extra.ins.sync_info = mybir.SyncInfo(on_wait=[w], on_update=[])
    nc.all_engine_barrier()
    popped = nc._tile_sem_poison_stack.pop()
    assert popped is self._sem_poison
    nc.clear_and_free_semaphores(list(self.sems.allocated().values()))
    nc.all_engine_barrier()


tile.TileContext._drain_and_barrier = _patched_drain_and_barrier

F32 = mybir.dt.float32
AF = mybir.ActivationFunctionType
OP = mybir.AluOpType

B, D, H = 2048, 64, 512
NCORES = 8
BS = B // NCORES  # 256 samples per core
NCH = H // 128    # 4 feature chunks of 128


def _build_program():
    nc = bass.Bass()

    xaT = nc.declare_dram_parameter("xaT", [D + 1, BS], F32, isOutput=False)
    w1 = nc.declare_dram_parameter("w1", [D + 1, H], F32, isOutput=False)
    b1 = nc.declare_dram_parameter("b1", [128, NCH], F32, isOutput=False)
    w2 = nc.declare_dram_parameter("w2", [H, H], F32, isOutput=False)
    b2 = nc.declare_dram_parameter("b2", [128, NCH], F32, isOutput=False)
    w3 = nc.declare_dram_parameter("w3", [H, D], F32, isOutput=False)
    km = nc.declare_dram_parameter("km", [H, H], F32, isOutput=False)
    b3 = nc.declare_dram_parameter("b3", [D, 1], F32, isOutput=False)
    out_dx = nc.declare_dram_parameter("out_dx", [D, BS], F32, isOutput=True)
    out_dv = nc.declare_dram_parameter("out_dv", [1, BS], F32, isOutput=True)

    with tile.TileContext(nc) as tc:
        with (
            tc.tile_pool(name="wts", bufs=1) as wts,
            tc.tile_pool(name="acts", bufs=1) as acts,
            tc.tile_pool(name="ps_z", bufs=3, space="PSUM") as ps_z,
            tc.tile_pool(name="ps_a", bufs=2, space="PSUM") as ps_a,
            tc.tile_pool(name="ps_o", bufs=1, space="PSUM") as ps_o,
        ):
            # ---- loads -------------------------------------------------
            xaT_sb = wts.tile([D + 1, BS], F32, tag="xaT_sb")
            nc.sync.dma_start(out=xaT_sb, in_=xaT[:, :])
            w1_sb = wts.tile([D + 1, H], F32, tag="w1_sb")
            nc.sync.dma_start(out=w1_sb, in_=w1[:, :])
            b1_sb = wts.tile([128, NCH], F32, tag="b1_sb")
            nc.sync.dma_start(out=b1_sb, in_=b1[:, :])
            b2_sb = wts.tile([128, NCH], F32, tag="b2_sb")
            nc.sync.dma_start(out=b2_sb, in_=b2[:, :])
            b3_sb = wts.tile([D, 1], F32, tag="b3_sb")
            nc.sync.dma_start(out=b3_sb, in_=b3[:, :])

            w2_sb, w3_sb, km_sb = [], [], []
            for c in range(NCH):
                w2_c = wts.tile([128, H], F32, tag=f"w2_{c}", name=f"w2c{c}")
                nc.sync.dma_start(out=w2_c, in_=w2[128 * c : 128 * (c + 1), :])
                w2_sb.append(w2_c)
                w3_c = wts.tile([128, D], F32, tag=f"w3_{c}", name=f"w3c{c}")
                nc.sync.dma_start(out=w3_c, in_=w3[128 * c : 128 * (c + 1), :])
                w3_sb.append(w3_c)
                km_c = wts.tile([128, H], F32, tag=f"km_{c}", name=f"kmc{c}")
                nc.sync.dma_start(out=km_c, in_=km[128 * c : 128 * (c + 1), :])
                km_sb.append(km_c)

            ones_sb = wts.tile([128, 1], F32, tag="ones_sb")
            nc.vector.memset(ones_sb, 1.0)

            # ---- layer 1: h1^T = tanh(W1^T @ xaT + b1) ------------------
            h1_sb, d1_sb = [], []
            for c in range(NCH):
                z1 = ps_z.tile([128, BS], F32, tag="z", name=f"z1_{c}")
                nc.tensor.matmul(
                    z1, lhsT=w1_sb[:, 128 * c : 128 * (c + 1)], rhs=xaT_sb,
                    start=True, stop=True,
                )
                h1 = acts.tile([128, BS], F32, tag=f"h1_{c}", name=f"h1_{c}")
                nc.scalar.activation(h1, z1, AF.Tanh, bias=b1_sb[:, c : c + 1])
                h1_sb.append(h1)

            # ---- layer 2: h2^T = tanh(W2^T @ h1^T + b2) -----------------
            h2_sb, d2_sb = [], []
            for c in range(NCH):
                z2 = ps_z.tile([128, BS], F32, tag="z", name=f"z2_{c}")
                for k in range(NCH):
                    nc.tensor.matmul(
                        z2, lhsT=w2_sb[k][:, 128 * c : 128 * (c + 1)], rhs=h1_sb[k],
                        start=(k == 0), stop=(k == NCH - 1),
                    )
                h2 = acts.tile([128, BS], F32, tag=f"h2_{c}", name=f"h2_{c}")
                nc.scalar.activation(h2, z2, AF.Tanh, bias=b2_sb[:, c : c + 1])
                h2_sb.append(h2)

            # ---- d = 1 - h^2 for both layers ----------------------------
            # squares: 2 on ACT (same table set as Tanh), 6 on DVE;
            # the affine 1-x on GPSIMD (single-input, SBUF-only).
            for li, (h_list, d_list) in enumerate(
                ((h1_sb, d1_sb), (h2_sb, d2_sb))
            ):
                for c in range(NCH):
                    hsq = acts.tile(
                        [128, BS], F32, tag="hsq", bufs=3, name=f"hsq{li}_{c}"
                    )
                    if li == 0 and c >= 2:
                        nc.scalar.activation(hsq, h_list[c], AF.Square)
                    else:
                        nc.vector.tensor_mul(hsq, h_list[c], h_list[c])
                    d = acts.tile([128, BS], F32, tag=f"d{li}_{c}", name=f"d{li}_{c}")
                    nc.gpsimd.tensor_scalar(
                        out=d, in0=hsq, scalar1=-1.0, scalar2=1.0,
                        op0=OP.mult, op1=OP.add,
                    )
                    d_list.append(d)

            # ---- layer 3: dx^T = (W3^T @ h2^T + b3) / 2 -----------------
            dx_ps = ps_o.tile([D, BS], F32, tag="dx")
            for k in range(NCH):
                nc.tensor.matmul(
                    dx_ps, lhsT=w3_sb[k], rhs=h2_sb[k],
                    start=(k == 0), stop=(k == NCH - 1),
                )
            dx_out = acts.tile([D, BS], F32, tag="dx_out")
            nc.vector.tensor_scalar(
                out=dx_out, in0=dx_ps, scalar1=b3_sb, scalar2=0.5,
                op0=OP.add, op1=OP.mult,
            )
            nc.sync.dma_start(out=out_dx[:, :], in_=dx_out)

            # ---- divergence: div = (d1^T K d2) / 2 ----------------------
            # A^T[j,b] = sum_m K[m,j] d1^T[m,b]  (K is lhsT-native)
            p_sb = []
            for c in range(NCH):
                a_ps = ps_a.tile([128, BS], F32, tag="a", name=f"a_{c}")
                for k in range(NCH):
                    nc.tensor.matmul(
                        a_ps, lhsT=km_sb[k][:, 128 * c : 128 * (c + 1)], rhs=d1_sb[k],
                        start=(k == 0), stop=(k == NCH - 1),
                    )
                p = acts.tile([128, BS], F32, tag=f"p_{c}", name=f"p_{c}")
                nc.vector.tensor_mul(p, a_ps, d2_sb[c])
                p_sb.append(p)

            # partition-dim reduction via ones-vector matmul
            dv_ps = ps_o.tile([1, BS], F32, tag="dv")
            for c in range(NCH):
                nc.tensor.matmul(
                    dv_ps, lhsT=ones_sb, rhs=p_sb[c],
                    start=(c == 0), stop=(c == NCH - 1),
                )
            dv_out = acts.tile([1, BS], F32, tag="dv_out")
            nc.vector.tensor_scalar(
                out=dv_out, in0=dv_ps, scalar1=0.5, scalar2=None, op0=OP.mult,
            )
            nc.sync.dma_start(out=out_dv[:, :], in_=dv_out)

    return nc


_NC = None


def _get_program():
    global _NC
    if _NC is None:
        _NC = _build_program()
    return _NC


def _host_prep(t, x, W1, b1, W2, b2, W3, b3):
    """Shard + lay out inputs for the device program (host does layout only,
    plus the launch-invariant weight fold K)."""
    t = np.asarray(t, np.float32)
    x = np.asarray(x, np.float32)
    W1 = np.asarray(W1, np.float32)
    W2 = np.asarray(W2, np.float32)
    W3 = np.asarray(W3, np.float32)
    b1 = np.asarray(b1, np.float32)
    b2 = np.asarray(b2, np.float32)
    b3 = np.asarray(b3, np.float32)

    # transposed, time-augmented input: rows 0..63 = x^T, row 64 = t
    xaT = np.empty((D + 1, B), np.float32)
    xaT[:D] = x[:, :D].T
    xaT[D] = t[0]

    # weight fold: K[m,j] = W2[m,j] * (W1[:D]^T @ W3^T)[m,j]
    km = W2 * (W1[:D].T @ W3.T)

    common = {
        "w1": np.ascontiguousarray(W1),
        "b1": np.ascontiguousarray(b1.reshape(NCH, 128).T),
        "w2": np.ascontiguousarray(W2),
        "b2": np.ascontiguousarray(b2.reshape(NCH, 128).T),
        "w3": np.ascontiguousarray(W3),
        "km": np.ascontiguousarray(km.astype(np.float32)),
        "b3": np.ascontiguousarray(b3.reshape(D, 1)),
    }
    in_maps = []
    for c in range(NCORES):
        m = dict(common)
        m["xaT"] = np.ascontiguousarray(xaT[:, BS * c : BS * (c + 1)])
        in_maps.append(m)
    return in_maps


def kernel(t, x, W1, b1, W2, b2, W3, b3):
    nc = _get_program()
    in_maps = _host_prep(t, x, W1, b1, W2, b2, W3, b3)
    res = run_bass_kernel_spmd(nc, in_maps, core_ids=list(range(NCORES)))
    out = np.empty((B, D + 1), np.float32)
    for c in range(NCORES):
        sl = slice(BS * c, BS * (c + 1))
        out[sl, :D] = res.results[c]["out_dx"].T
        out[sl, D] = res.results[c]["out_dv"][0]
    return out


# revision 7
# speedup vs baseline: 1.2051x; 1.2051x over previous
"""CNF step (3-layer tanh MLP vector field + exact divergence) on 8 trn2 cores.

Math: for each sample x in R^64 (x's last column is the logp channel, replaced
by scalar t in the MLP input):
    h1 = tanh([x, t] @ W1 + b1);  h2 = tanh(h1 @ W2 + b2)
    dx = (h2 @ W3 + b3) / 2
    div = trace(J) where J = d(dx)/dx
Closed form for the jacobian trace (avoids jacrev entirely):
    div = (1/2) * d1^T K d2,  d1 = 1-h1^2, d2 = 1-h2^2,
    K[m,j] = W2[m,j] * sum_i W1[i,m] W3[j,i]
K is a pure function of the (launch-invariant) weights, folded on host once.
All O(batch) compute runs on device.

Device layout is fully transposed (features on partitions, batch on free dim):
weights serve directly as matmul lhsT operands, so the kernel needs zero
on-device transposes. Host pre-transposes x (layout prep) and re-transposes
the output.

Sharding: pure data parallel, batch 2048 -> 8 cores x 256 samples.
"""

import numpy as np

import bass_rust
import concourse.bass as bass
import concourse.tile as tile
from concourse import mybir
from concourse.bass_utils import run_bass_kernel_spmd

# This walrus build only encodes a single sem-wait per instruction; Tile's
# scheduler freely emits instructions carrying 2-3 waits and codegen dies
# with "Too many sync wait commands". Two patches hoist extra waits onto
# single-wait EventSemaphore carrier instructions placed immediately before
# the multi-wait instruction on the same engine (semantically identical:
# engines execute in order, all waits still precede the op).
_orig_add_instruction = tile.TileContext._add_instruction


def _split_waits(tc_self, inst):
    si = getattr(inst, "sync_info", None)
    if (
        si is not None
        and si.on_wait
        and len(si.on_wait) > 1
        and inst.engine != mybir.EngineType.Unassigned
    ):
        waits = list(si.on_wait)
        upds = list(si.on_update) if si.on_update else []
        for w in waits[:-1]:
            carrier = mybir.InstEventSemaphore(
                name=tc_self.nc.get_next_instruction_name(),
                engine=inst.engine,
                ins=[],
                outs=[],
                sync_info=mybir.SyncInfo(on_wait=[w], on_update=[]),
                bass_nofuse=True,
            )
            _orig_add_instruction(tc_self, carrier)
        inst.sync_info = mybir.SyncInfo(on_wait=[waits[-1]], on_update=upds)


def _patched_add_instruction(self, inst):
    _split_waits(self, inst)
    _orig_add_instruction(self, inst)


tile.TileContext._add_instruction = _patched_add_instruction


# Same fix for the kernel-tail drain (emitted after lowering, outside
# _add_instruction): split it into a chain of single-wait drains.
def _patched_drain_and_barrier(self, tick_clock, wait_clock):
    nc = self.nc
    drain_inst = nc.sync.drain()
    wait_clock.add_sem_waits(
        drain_inst.ins, bass_rust.ScopedClock({None: tick_clock.global_clock})
    )
    si = drain_inst.ins.sync_info
    waits = list(si.on_wait) if si is not None and si.on_wait else []
    if len(waits) > 1:
        upds = list(si.on_update) if si.on_update else []
        drain_inst.ins.sync_info = mybir.SyncInfo(on_wait=[waits[0]], on_update=upds)
        for w in waits[1:]:
            extra = nc.sync.drain()
            extra.ins.sync_info = mybir.SyncInfo(on_wait=[w], on_update=[])
    nc.all_engine_barrier()
    popped = nc._tile_sem_poison_stack.pop()
    assert popped is self._sem_poison
    nc.clear_and_free_semaphores(list(self.sems.allocated().values()))
    nc.all_engine_barrier()


tile.TileContext._drain_and_barrier = _patched_drain_and_barrier

F32 = mybir.dt.float32
F32R = mybir.dt.float32r
AF = mybir.ActivationFunctionType
OP = mybir.AluOpType

B, D, H = 2048, 64, 512
NCORES = 8
BS = B // NCORES  # 256 samples per core
NCH = H // 128    # 4 feature chunks of 128


def _build_program():
    nc = bass.Bass()

    xaT = nc.declare_dram_parameter("xaT", [D + 1, BS], F32, isOutput=False)
    w1 = nc.declare_dram_parameter("w1", [D + 1, H], F32, isOutput=False)
    b1 = nc.declare_dram_parameter("b1", [128, NCH], F32, isOutput=False)
    w2 = nc.declare_dram_parameter("w2", [H, H], F32, isOutput=False)
    b2 = nc.declare_dram_parameter("b2", [128, NCH], F32, isOutput=False)
    w3 = nc.declare_dram_parameter("w3", [H, D], F32, isOutput=False)
    km = nc.declare_dram_parameter("km", [H, H], F32, isOutput=False)
    b3 = nc.declare_dram_parameter("b3", [D, 1], F32, isOutput=False)
    ones = nc.declare_dram_parameter("ones", [128, 1], F32, isOutput=False)
    out_dx = nc.declare_dram_parameter("out_dx", [D, BS], F32, isOutput=True)
    out_dv = nc.declare_dram_parameter("out_dv", [1, BS], F32, isOutput=True)

    with tile.TileContext(nc) as tc:
        with (
            tc.tile_pool(name="wts", bufs=1) as wts,
            tc.tile_pool(name="acts", bufs=1) as acts,
            tc.tile_pool(name="ps_z", bufs=3, space="PSUM") as ps_z,
            tc.tile_pool(name="ps_a", bufs=2, space="PSUM") as ps_a,
            tc.tile_pool(name="ps_o", bufs=1, space="PSUM") as ps_o,
        ):
            # ---- loads -------------------------------------------------
            xaT_sb = wts.tile([D + 1, BS], F32, tag="xaT_sb")
            nc.sync.dma_start(out=xaT_sb.bitcast(F32R), in_=xaT[:, :].bitcast(F32R))
            w1_sb = wts.tile([D + 1, H], F32, tag="w1_sb")
            nc.sync.dma_start(out=w1_sb.bitcast(F32R), in_=w1[:, :].bitcast(F32R))
            b1_sb = wts.tile([128, NCH], F32, tag="b1_sb")
            nc.sync.dma_start(out=b1_sb, in_=b1[:, :])
            b2_sb = wts.tile([128, NCH], F32, tag="b2_sb")
            nc.sync.dma_start(out=b2_sb, in_=b2[:, :])
            b3_sb = wts.tile([D, 1], F32, tag="b3_sb")
            nc.sync.dma_start(out=b3_sb, in_=b3[:, :])

            w2_sb, w3_sb, km_sb = [], [], []
            for c in range(NCH):
                w2_c = wts.tile([128, H], F32, tag=f"w2_{c}", name=f"w2c{c}")
                nc.sync.dma_start(out=w2_c.bitcast(F32R), in_=w2[128 * c : 128 * (c + 1), :].bitcast(F32R))
                w2_sb.append(w2_c)
                w3_c = wts.tile([128, D], F32, tag=f"w3_{c}", name=f"w3c{c}")
                nc.sync.dma_start(out=w3_c.bitcast(F32R), in_=w3[128 * c : 128 * (c + 1), :].bitcast(F32R))
                w3_sb.append(w3_c)
                km_c = wts.tile([128, H], F32, tag=f"km_{c}", name=f"kmc{c}")
                nc.sync.dma_start(out=km_c.bitcast(F32R), in_=km[128 * c : 128 * (c + 1), :].bitcast(F32R))
                km_sb.append(km_c)

            ones_sb = wts.tile([128, 1], F32, tag="ones_sb")
            nc.sync.dma_start(out=ones_sb.bitcast(F32R), in_=ones[:, :].bitcast(F32R))

            # ---- layer 1: h1^T = tanh(W1^T @ xaT + b1) ------------------
            h1_sb, d1_sb = [], []
            for c in range(NCH):
                z1 = ps_z.tile([128, BS], F32, tag="z", name=f"z1_{c}")
                nc.tensor.matmul(
                    z1, lhsT=w1_sb[:, 128 * c : 128 * (c + 1)].bitcast(F32R), rhs=xaT_sb.bitcast(F32R),
                    start=True, stop=True,
                )
                h1 = acts.tile([128, BS], F32, tag=f"h1_{c}", name=f"h1_{c}")
                nc.scalar.activation(h1.bitcast(F32R), z1, AF.Tanh, bias=b1_sb[:, c : c + 1])
                h1_sb.append(h1)

            # ---- layer 2: h2^T = tanh(W2^T @ h1^T + b2) -----------------
            h2_sb, d2_sb = [], []
            for c in range(NCH):
                z2 = ps_z.tile([128, BS], F32, tag="z", name=f"z2_{c}")
                for k in range(NCH):
                    nc.tensor.matmul(
                        z2, lhsT=w2_sb[k][:, 128 * c : 128 * (c + 1)].bitcast(F32R), rhs=h1_sb[k].bitcast(F32R),
                        start=(k == 0), stop=(k == NCH - 1),
                    )
                h2 = acts.tile([128, BS], F32, tag=f"h2_{c}", name=f"h2_{c}")
                nc.scalar.activation(h2.bitcast(F32R), z2, AF.Tanh, bias=b2_sb[:, c : c + 1])
                h2_sb.append(h2)

            # ---- d = 1 - h^2 for both layers ----------------------------
            # squares: 2 on ACT (same table set as Tanh), 6 on DVE;
            # the affine 1-x on GPSIMD (single-input, SBUF-only).
            for li, (h_list, d_list) in enumerate(
                ((h1_sb, d1_sb), (h2_sb, d2_sb))
            ):
                for c in range(NCH):
                    hsq = acts.tile(
                        [128, BS], F32, tag="hsq", bufs=3, name=f"hsq{li}_{c}"
                    )
                    if li == 0 and c >= 2:
                        nc.scalar.activation(hsq, h_list[c], AF.Square)
                    else:
                        nc.vector.tensor_mul(hsq, h_list[c], h_list[c])
                    d = acts.tile([128, BS], F32, tag=f"d{li}_{c}", name=f"d{li}_{c}")
                    nc.gpsimd.tensor_scalar(
                        out=d.bitcast(F32R), in0=hsq, scalar1=-1.0, scalar2=1.0,
                        op0=OP.mult, op1=OP.add,
                    )
                    d_list.append(d)

            # ---- layer 3: dx^T = (W3^T @ h2^T + b3) / 2 -----------------
            dx_ps = ps_o.tile([D, BS], F32, tag="dx")
            for k in range(NCH):
                nc.tensor.matmul(
                    dx_ps, lhsT=w3_sb[k].bitcast(F32R), rhs=h2_sb[k].bitcast(F32R),
                    start=(k == 0), stop=(k == NCH - 1),
                )
            dx_out = acts.tile([D, BS], F32, tag="dx_out")
            nc.vector.tensor_scalar(
                out=dx_out, in0=dx_ps, scalar1=b3_sb, scalar2=0.5,
                op0=OP.add, op1=OP.mult,
            )
            nc.sync.dma_start(out=out_dx[:, :], in_=dx_out)

            # ---- divergence: div = (d1^T K d2) / 2 ----------------------
            # A^T[j,b] = sum_m K[m,j] d1^T[m,b]  (K is lhsT-native)
            p_sb = []
            for c in range(NCH):
                a_ps = ps_a.tile([128, BS], F32, tag="a", name=f"a_{c}")
                for k in range(NCH):
                    nc.tensor.matmul(
                        a_ps, lhsT=km_sb[k][:, 128 * c : 128 * (c + 1)].bitcast(F32R), rhs=d1_sb[k].bitcast(F32R),
                        start=(k == 0), stop=(k == NCH - 1),
                    )
                p = acts.tile([128, BS], F32, tag=f"p_{c}", name=f"p_{c}")
                nc.vector.tensor_mul(p.bitcast(F32R), a_ps, d2_sb[c])
                p_sb.append(p)

            # partition-dim reduction via ones-vector matmul
            dv_ps = ps_o.tile([1, BS], F32, tag="dv")
            for c in range(NCH):
                nc.tensor.matmul(
                    dv_ps, lhsT=ones_sb.bitcast(F32R), rhs=p_sb[c].bitcast(F32R),
                    start=(c == 0), stop=(c == NCH - 1),
                )
            dv_out = acts.tile([1, BS], F32, tag="dv_out")
            nc.vector.tensor_scalar(
                out=dv_out, in0=dv_ps, scalar1=0.5, scalar2=None, op0=OP.mult,
            )
            nc.sync.dma_start(out=out_dv[:, :], in_=dv_out)

    return nc


_NC = None


def _get_program():
    global _NC
    if _NC is None:
        _NC = _build_program()
    return _NC


def _host_prep(t, x, W1, b1, W2, b2, W3, b3):
    """Shard + lay out inputs for the device program (host does layout only,
    plus the launch-invariant weight fold K)."""
    t = np.asarray(t, np.float32)
    x = np.asarray(x, np.float32)
    W1 = np.asarray(W1, np.float32)
    W2 = np.asarray(W2, np.float32)
    W3 = np.asarray(W3, np.float32)
    b1 = np.asarray(b1, np.float32)
    b2 = np.asarray(b2, np.float32)
    b3 = np.asarray(b3, np.float32)

    # transposed, time-augmented input: rows 0..63 = x^T, row 64 = t
    xaT = np.empty((D + 1, B), np.float32)
    xaT[:D] = x[:, :D].T
    xaT[D] = t[0]

    # weight fold: K[m,j] = W2[m,j] * (W1[:D]^T @ W3^T)[m,j]
    km = W2 * (W1[:D].T @ W3.T)

    common = {
        "w1": np.ascontiguousarray(W1),
        "b1": np.ascontiguousarray(b1.reshape(NCH, 128).T),
        "w2": np.ascontiguousarray(W2),
        "b2": np.ascontiguousarray(b2.reshape(NCH, 128).T),
        "w3": np.ascontiguousarray(W3),
        "km": np.ascontiguousarray(km.astype(np.float32)),
        "b3": np.ascontiguousarray(b3.reshape(D, 1)),
        "ones": np.ones((128, 1), np.float32),
    }
    in_maps = []
    for c in range(NCORES):
        m = dict(common)
        m["xaT"] = np.ascontiguousarray(xaT[:, BS * c : BS * (c + 1)])
        in_maps.append(m)
    return in_maps


def kernel(t, x, W1, b1, W2, b2, W3, b3):
    nc = _get_program()
    in_maps = _host_prep(t, x, W1, b1, W2, b2, W3, b3)
    res = run_bass_kernel_spmd(nc, in_maps, core_ids=list(range(NCORES)))
    out = np.empty((B, D + 1), np.float32)
    for c in range(NCORES):
        sl = slice(BS * c, BS * (c + 1))
        out[sl, :D] = res.results[c]["out_dx"].T
        out[sl, D] = res.results[c]["out_dv"][0]
    return out


# revision 9
# speedup vs baseline: 1.2533x; 1.0400x over previous
"""CNF step (3-layer tanh MLP vector field + exact divergence) on 8 trn2 cores.

Math: for each sample x in R^64 (x's last column is the logp channel, replaced
by scalar t in the MLP input):
    h1 = tanh([x, t] @ W1 + b1);  h2 = tanh(h1 @ W2 + b2)
    dx = (h2 @ W3 + b3) / 2
    div = trace(J) where J = d(dx)/dx
Closed form for the jacobian trace (avoids jacrev entirely):
    div = (1/2) * d1^T K d2,  d1 = 1-h1^2, d2 = 1-h2^2,
    K[m,j] = W2[m,j] * sum_i W1[i,m] W3[j,i]
K is a pure function of the (launch-invariant) weights, folded on host once.
All O(batch) compute runs on device.

Device layout is fully transposed (features on partitions, batch on free dim):
weights serve directly as matmul lhsT operands, so the kernel needs zero
on-device transposes. Host pre-transposes x (layout prep) and re-transposes
the output. Matmuls run in float32r (single-pass, full-rate) -- every producer
of a matmul operand writes f32r explicitly, as the BIR verifier requires.

Sharding: pure data parallel, batch 2048 -> 8 cores x 256 samples.
"""

import numpy as np

import bass_rust
import concourse.bass as bass
import concourse.tile as tile
from concourse import mybir
from concourse.bass_utils import run_bass_kernel_spmd

# This walrus build only encodes a single sem-wait per instruction; Tile's
# scheduler freely emits instructions carrying 2-3 waits and codegen dies
# with "Too many sync wait commands". Hoist extra waits onto single-wait
# EventSemaphore carrier instructions placed immediately before the
# multi-wait instruction on the same engine (semantically identical:
# engines execute in order, all waits still precede the op).
_orig_add_instruction = tile.TileContext._add_instruction


def _split_waits(tc_self, inst):
    si = getattr(inst, "sync_info", None)
    if (
        si is not None
        and si.on_wait
        and len(si.on_wait) > 1
        and inst.engine != mybir.EngineType.Unassigned
    ):
        waits = list(si.on_wait)
        upds = list(si.on_update) if si.on_update else []
        for w in waits[:-1]:
            carrier = mybir.InstEventSemaphore(
                name=tc_self.nc.get_next_instruction_name(),
                engine=inst.engine,
                ins=[],
                outs=[],
                sync_info=mybir.SyncInfo(on_wait=[w], on_update=[]),
                bass_nofuse=True,
            )
            _orig_add_instruction(tc_self, carrier)
        inst.sync_info = mybir.SyncInfo(on_wait=[waits[-1]], on_update=upds)


def _patched_add_instruction(self, inst):
    _split_waits(self, inst)
    _orig_add_instruction(self, inst)


tile.TileContext._add_instruction = _patched_add_instruction


# Minimal kernel tail. Tile's stock tail (drain + all-engine barrier + sem
# clear + barrier) exists to reset semaphore/DMA state for the next
# execution -- but the Bass preamble at the START of every execution already
# clears the whole kernel sem range (range(150,256)) and resets DMA state,
# so the tail only needs to hold the NEFF open until every outstanding sem
# (including the output-store DMA completions) reaches its terminal value.
# Emit that as a chain of single-wait drains on SP (the walrus build's
# 1-wait-per-instruction limit again).
def _patched_drain_and_barrier(self, tick_clock, wait_clock):
    nc = self.nc
    drain_inst = nc.sync.drain()
    wait_clock.add_sem_waits(
        drain_inst.ins, bass_rust.ScopedClock({None: tick_clock.global_clock})
    )
    si = drain_inst.ins.sync_info
    waits = list(si.on_wait) if si is not None and si.on_wait else []
    if len(waits) > 1:
        upds = list(si.on_update) if si.on_update else []
        drain_inst.ins.sync_info = mybir.SyncInfo(on_wait=[waits[0]], on_update=upds)
        for w in waits[1:]:
            extra = nc.sync.drain()
            extra.ins.sync_info = mybir.SyncInfo(on_wait=[w], on_update=[])
    popped = nc._tile_sem_poison_stack.pop()
    assert popped is self._sem_poison


tile.TileContext._drain_and_barrier = _patched_drain_and_barrier

F32 = mybir.dt.float32
F32R = mybir.dt.float32r
AF = mybir.ActivationFunctionType
OP = mybir.AluOpType

B, D, H = 2048, 64, 512
NCORES = 8
BS = B // NCORES  # 256 samples per core
NCH = H // 128    # 4 feature chunks of 128


def _build_program():
    nc = bass.Bass()

    xaT = nc.declare_dram_parameter("xaT", [D + 1, BS], F32, isOutput=False)
    w1 = nc.declare_dram_parameter("w1", [D + 1, H], F32, isOutput=False)
    w2 = nc.declare_dram_parameter("w2", [H, H], F32, isOutput=False)
    w3 = nc.declare_dram_parameter("w3", [H, D], F32, isOutput=False)
    km = nc.declare_dram_parameter("km", [H, H], F32, isOutput=False)
    b12 = nc.declare_dram_parameter("b12", [128, 2 * NCH], F32, isOutput=False)
    b3o = nc.declare_dram_parameter("b3o", [128, 2], F32, isOutput=False)
    out_dx = nc.declare_dram_parameter("out_dx", [D, BS], F32, isOutput=True)
    out_dv = nc.declare_dram_parameter("out_dv", [1, BS], F32, isOutput=True)

    w2r = w2.rearrange("(c p) j -> p c j", p=128)  # [128, 4, 512]
    kmr = km.rearrange("(c p) j -> p c j", p=128)
    w3r = w3.rearrange("(c p) j -> p c j", p=128)  # [128, 4, 64]

    with tile.TileContext(nc) as tc:
        with (
            tc.tile_pool(name="wts", bufs=1) as wts,
            tc.tile_pool(name="acts", bufs=1) as acts,
            tc.tile_pool(name="ps_z", bufs=3, space="PSUM") as ps_z,
            tc.tile_pool(name="ps_a", bufs=2, space="PSUM") as ps_a,
            tc.tile_pool(name="ps_o", bufs=1, space="PSUM") as ps_o,
        ):
            # ---- loads: spread across engine DMA queues -----------------
            xaT_sb = wts.tile([D + 1, BS], F32, tag="xaT_sb")
            nc.sync.dma_start(out=xaT_sb.bitcast(F32R), in_=xaT[:, :].bitcast(F32R))
            w1_sb = wts.tile([D + 1, H], F32, tag="w1_sb")
            nc.sync.dma_start(out=w1_sb.bitcast(F32R), in_=w1[:, :].bitcast(F32R))
            b12_sb = wts.tile([128, 2 * NCH], F32, tag="b12_sb")
            nc.sync.dma_start(out=b12_sb, in_=b12[:, :])
            b3o_sb = wts.tile([128, 2], F32, tag="b3o_sb")
            nc.sync.dma_start(out=b3o_sb.bitcast(F32R), in_=b3o[:, :].bitcast(F32R))
            w3_sb = wts.tile([128, NCH, D], F32, tag="w3_sb")
            nc.sync.dma_start(out=w3_sb.bitcast(F32R), in_=w3r.bitcast(F32R))

            w2_sb = wts.tile([128, NCH, H], F32, tag="w2_sb")
            nc.scalar.dma_start(out=w2_sb.bitcast(F32R), in_=w2r.bitcast(F32R))
            km_sb = wts.tile([128, NCH, H], F32, tag="km_sb")
            nc.gpsimd.dma_start(out=km_sb.bitcast(F32R), in_=kmr.bitcast(F32R))

            # ---- layer 1: h1^T = tanh(W1^T @ xaT + b1) ------------------
            h1_sb = acts.tile([128, NCH * BS], F32, tag="h1_sb")
            for c in range(NCH):
                z1 = ps_z.tile([128, BS], F32, tag="z", name=f"z1_{c}")
                nc.tensor.matmul(
                    z1,
                    lhsT=w1_sb[:, 128 * c : 128 * (c + 1)].bitcast(F32R),
                    rhs=xaT_sb.bitcast(F32R),
                    start=True,
                    stop=True,
                )
                nc.scalar.activation(
                    h1_sb[:, BS * c : BS * (c + 1)].bitcast(F32R),
                    z1,
                    AF.Tanh,
                    bias=b12_sb[:, c : c + 1],
                )

            # ---- layer 2: h2^T = tanh(W2^T @ h1^T + b2) -----------------
            h2_sb = acts.tile([128, NCH * BS], F32, tag="h2_sb")
            for c in range(NCH):
                z2 = ps_z.tile([128, BS], F32, tag="z", name=f"z2_{c}")
                for k in range(NCH):
                    nc.tensor.matmul(
                        z2,
                        lhsT=w2_sb[:, k, 128 * c : 128 * (c + 1)].bitcast(F32R),
                        rhs=h1_sb[:, BS * k : BS * (k + 1)].bitcast(F32R),
                        start=(k == 0),
                        stop=(k == NCH - 1),
                    )
                nc.scalar.activation(
                    h2_sb[:, BS * c : BS * (c + 1)].bitcast(F32R),
                    z2,
                    AF.Tanh,
                    bias=b12_sb[:, NCH + c : NCH + c + 1],
                )

            # ---- d = 1 - h^2 (wide ops; squares split ACT/DVE) ----------
            hsq1 = acts.tile([128, NCH * BS], F32, tag="hsq1")
            nc.scalar.activation(hsq1, h1_sb, AF.Square)
            d1_sb = acts.tile([128, NCH * BS], F32, tag="d1_sb")
            nc.vector.tensor_scalar(
                out=d1_sb.bitcast(F32R), in0=hsq1, scalar1=-1.0, scalar2=1.0,
                op0=OP.mult, op1=OP.add,
            )
            hsq2 = acts.tile([128, NCH * BS], F32, tag="hsq2")
            nc.vector.tensor_mul(hsq2, h2_sb, h2_sb)
            d2_sb = acts.tile([128, NCH * BS], F32, tag="d2_sb")
            nc.vector.tensor_scalar(
                out=d2_sb, in0=hsq2, scalar1=-1.0, scalar2=1.0,
                op0=OP.mult, op1=OP.add,
            )

            # ---- layer 3: dx^T = (W3^T @ h2^T + b3) / 2 -----------------
            dx_ps = ps_o.tile([D, BS], F32, tag="dx")
            for k in range(NCH):
                nc.tensor.matmul(
                    dx_ps,
                    lhsT=w3_sb[:, k, :].bitcast(F32R),
                    rhs=h2_sb[:, BS * k : BS * (k + 1)].bitcast(F32R),
                    start=(k == 0),
                    stop=(k == NCH - 1),
                )
            dx_out = acts.tile([D, BS], F32, tag="dx_out")
            nc.vector.tensor_scalar(
                out=dx_out, in0=dx_ps, scalar1=b3o_sb[0:D, 0:1], scalar2=0.5,
                op0=OP.add, op1=OP.mult,
            )
            nc.sync.dma_start(out=out_dx[:, :], in_=dx_out)

            # ---- divergence: div = (d1^T K d2) / 2 ----------------------
            # A^T[j,b] = sum_m K[m,j] d1^T[m,b]  (K is lhsT-native)
            p_sb = acts.tile([128, NCH * BS], F32, tag="p_sb")
            for c in range(NCH):
                a_ps = ps_a.tile([128, BS], F32, tag="a", name=f"a_{c}")
                for k in range(NCH):
                    nc.tensor.matmul(
                        a_ps,
                        lhsT=km_sb[:, k, 128 * c : 128 * (c + 1)].bitcast(F32R),
                        rhs=d1_sb[:, BS * k : BS * (k + 1)].bitcast(F32R),
                        start=(k == 0),
                        stop=(k == NCH - 1),
                    )
                nc.vector.tensor_mul(
                    p_sb[:, BS * c : BS * (c + 1)].bitcast(F32R),
                    a_ps,
                    d2_sb[:, BS * c : BS * (c + 1)],
                )

            # partition-dim reduction via ones-vector matmul
            dv_ps = ps_o.tile([1, BS], F32, tag="dv")
            for c in range(NCH):
                nc.tensor.matmul(
                    dv_ps,
                    lhsT=b3o_sb[:, 1:2].bitcast(F32R),
                    rhs=p_sb[:, BS * c : BS * (c + 1)].bitcast(F32R),
                    start=(c == 0),
                    stop=(c == NCH - 1),
                )
            dv_out = acts.tile([1, BS], F32, tag="dv_out")
            nc.vector.tensor_scalar(
                out=dv_out, in0=dv_ps, scalar1=0.5, scalar2=None, op0=OP.mult,
            )
            nc.sync.dma_start(out=out_dv[:, :], in_=dv_out)

    return nc


_NC = None


def _get_program():
    global _NC
    if _NC is None:
        _NC = _build_program()
    return _NC


def _host_prep(t, x, W1, b1, W2, b2, W3, b3):
    """Shard + lay out inputs for the device program (host does layout only,
    plus the launch-invariant weight fold K)."""
    t = np.asarray(t, np.float32)
    x = np.asarray(x, np.float32)
    W1 = np.asarray(W1, np.float32)
    W2 = np.asarray(W2, np.float32)
    W3 = np.asarray(W3, np.float32)
    b1 = np.asarray(b1, np.float32)
    b2 = np.asarray(b2, np.float32)
    b3 = np.asarray(b3, np.float32)

    # transposed, time-augmented input: rows 0..63 = x^T, row 64 = t
    xaT = np.empty((D + 1, B), np.float32)
    xaT[:D] = x[:, :D].T
    xaT[D] = t[0]

    # weight fold: K[m,j] = W2[m,j] * (W1[:D]^T @ W3^T)[m,j]
    km = W2 * (W1[:D].T @ W3.T)

    # biases packed per 128-partition chunk: cols 0..3 = b1, 4..7 = b2
    b12 = np.concatenate(
        [b1.reshape(NCH, 128).T, b2.reshape(NCH, 128).T], axis=1
    )
    # col 0 = b3 (rows 0..63), col 1 = ones (for the partition reduction)
    b3o = np.zeros((128, 2), np.float32)
    b3o[:D, 0] = b3
    b3o[:, 1] = 1.0

    common = {
        "w1": np.ascontiguousarray(W1),
        "w2": np.ascontiguousarray(W2),
        "w3": np.ascontiguousarray(W3),
        "km": np.ascontiguousarray(km.astype(np.float32)),
        "b12": np.ascontiguousarray(b12),
        "b3o": b3o,
    }
    in_maps = []
    for c in range(NCORES):
        m = dict(common)
        m["xaT"] = np.ascontiguousarray(xaT[:, BS * c : BS * (c + 1)])
        in_maps.append(m)
    return in_maps


def kernel(t, x, W1, b1, W2, b2, W3, b3):
    nc = _get_program()
    in_maps = _host_prep(t, x, W1, b1, W2, b2, W3, b3)
    res = run_bass_kernel_spmd(nc, in_maps, core_ids=list(range(NCORES)))
    out = np.empty((B, D + 1), np.float32)
    for c in range(NCORES):
        sl = slice(BS * c, BS * (c + 1))
        out[sl, :D] = res.results[c]["out_dx"].T
        out[sl, D] = res.results[c]["out_dv"][0]
    return out
